# revision 27
# baseline (speedup 1.0000x reference)
"""Trainium2 Bass kernel for the HGCA contrastive loss (nn_HGCA_10857677324785).

loss = mean_i 0.5*(l1_i + l2_i),
  l1_i = log(den1_i) - log(num_i), den1_i = sum_j e^{2 an_i.an_j} + sum_j e^{2 an_i.bn_j} - e^2
  l2_i = log(den2_i) - log(num_i), den2_i = sum_j e^{2 bn_i.bn_j} + sum_j e^{2 bn_i.an_j} - e^2
where an/bn are the L2-normalized projections elu-MLP(z1/z2).

Distribution: the projections (O(N D^2), 0.5% of the FLOPs) are computed on
the host in f32 — the sharding hint's "all-gathered normalized projections"
— and handed to every core pre-transposed ([d, i] layout, bf16) and
row-rolled so each core's 2048 rows sit at local columns [0, 2048).

Each core computes its row-block of the three N x N similarity exps.  The
symmetric matrices S11 = an@an.T and S22 = bn@bn.T are only half-computed:
each 128-row tile m processes a diagonal-anchored window of 64 column tiles
(local cols [m*128, m*128+8192)) plus the distance-64 tile as a separate
"band" pass.  Row sums come from the ACT engine's fused accumulator; column
sums of the D in [1,63] part are accumulated in bf16 on the DVE and exported
raw — by symmetry they are exactly the row-sum contributions of the
uncomputed distance >= 65 tiles.  S12 is not symmetric: full rows with both
row sums (ACT accum) and bf16 column accumulation (DVE).  The host sums the
raw column accumulators over partitions, rolls them into global row space,
and assembles the scalar loss in f64 (log num_i = 2 an_i.bn_i directly).
"""

import ml_dtypes
import numpy as np

import concourse.bass as bass
import concourse.tile as tile
from concourse import mybir
from concourse.bass_utils import run_bass_kernel_spmd

N = 16384
D = 128
NCORES = 8
R = N // NCORES  # 2048 rows per core
TILES = R // 128  # 16 row tiles per core
WIN = 8192  # window: distance tiles 0..63
CHUNK = 1024  # psum/exp sub-chunk width (2-bank PSUM tiles, 4-deep rotation)
PAIR = 2048  # column-accumulate granularity (hw limit for accum DMAs)
CA_COLS = 15 * 128 + WIN - 128  # 9984: colacc for D in [1,63]
ANT_COLS = 15 * 128 + WIN + 2048  # 10240: rightmost anT column ever read
INV_TAU = 2.0  # 1/0.5
F32 = mybir.dt.float32
BF16 = mybir.dt.bfloat16
I16 = mybir.dt.int16
AF = mybir.ActivationFunctionType
OP = mybir.AluOpType

# Schraudolph fast-exp on the DVE: I = int16(A*s + B); bf16-bits(I) ~ exp(2s).
# A folds in 1/tau; B calibrated for zero mean multiplicative bias over the
# (near-uniform) mantissa phase.  Offloads ACT-engine exp work per chunk.
SCHRA_A = 2.0 * 128.0 / float(np.log(2.0))
SCHRA_B = 16250.0
OFF_NUM, OFF_DEN = 223, 512  # fraction of sub-chunks exp'd on DVE (Bresenham)
DVE12_P = 1  # S12 column pairs p < this accumulate on DVE (SBUF)
POOL12_P = 3  # S12 pairs p in [DVE12_P, this) accumulate on gpsimd (SBUF)
# S12 pairs p >= POOL12_P go per-pair SWDGE-accumulate to DRAM

# This walrus build supports at most 2 sync waits per instruction; Tile's sem
# assignment freely emits 3-11. Post-pass: hoist excess waits onto injected
# same-engine EventSemaphore fillers (engine queues are FIFO, so waits on an
# earlier filler happen-before the original instruction executes).

_MAX_WAITS = 1


def _split_waits(nc):
    for fn in nc.m.functions:
        for bb in fn.blocks:
            insts = list(bb.instructions)
            out = []
            changed = False
            for inst in insts:
                si = inst.sync_info
                w = list(si.on_wait) if si and si.on_wait else []
                if len(w) > _MAX_WAITS:
                    changed = True
                    extra, keep = w[:-_MAX_WAITS], w[-_MAX_WAITS:]
                    for i in range(0, len(extra), _MAX_WAITS):
                        f = mybir.InstEventSemaphore(
                            name=f"{inst.name}_wsplit{i}",
                            engine=inst.engine,
                            ins=[],
                            outs=[],
                            sync_info=mybir.SyncInfo(
                                on_wait=extra[i : i + _MAX_WAITS], on_update=[]
                            ),
                        )
                        out.append(f)
                    inst.sync_info = mybir.SyncInfo(
                        on_wait=keep,
                        on_update=list(si.on_update) if si.on_update else [],
                    )
                out.append(inst)
            if changed:
                bb.instructions = out


def _patched_drain_and_barrier(self, tick_clock, wait_clock):
    from concourse.vector_clock import ScopedClock

    nc = self.nc
    drain_inst = nc.sync.drain()
    wait_clock.add_sem_waits(
        drain_inst.ins, ScopedClock({None: tick_clock.global_clock})
    )
    nc.all_engine_barrier()
    assert self.sems is not None
    popped = nc._tile_sem_poison_stack.pop()
    assert popped is self._sem_poison
    nc.clear_and_free_semaphores(list(self.sems.allocated().values()))
    nc.all_engine_barrier()
    _split_waits(nc)


tile.TileContext._drain_and_barrier = _patched_drain_and_barrier

_NC_CACHE = None
RUN_KWARGS: dict = {}
LAST_RES = None


def _build():
    nc = bass.Bass("TRN2", target_bir_lowering=False, debug=False)

    anT_d = nc.dram_tensor("anT", [128, ANT_COLS], BF16, kind="ExternalInput").ap()
    bnT_d = nc.dram_tensor("bnT", [128, N], BF16, kind="ExternalInput").ap()

    acc11_d = nc.dram_tensor("acc11", [128, 8 * TILES], F32, kind="ExternalOutput").ap()
    acc22_d = nc.dram_tensor("acc22", [128, 8 * TILES], F32, kind="ExternalOutput").ap()
    acc12_d = nc.dram_tensor("acc12", [128, 16 * TILES], F32, kind="ExternalOutput").ap()
    band11_d = nc.dram_tensor("band11", [128, R], BF16, kind="ExternalOutput").ap()
    band22_d = nc.dram_tensor("band22", [128, R], BF16, kind="ExternalOutput").ap()
    ca11_d = nc.dram_tensor("ca11", [128, CA_COLS], BF16, kind="ExternalOutput").ap()
    ca22_d = nc.dram_tensor("ca22", [128, CA_COLS], BF16, kind="ExternalOutput").ap()
    ca12_d = nc.dram_tensor("ca12", [128, N], BF16, kind="ExternalOutput").ap()

    with tile.TileContext(nc) as tc:
        with tc.tile_pool(name="pers", bufs=1) as pers:
            anT = pers.tile([128, ANT_COLS], BF16, tag="anT")
            bnT = pers.tile([128, N], BF16, tag="bnT")
            ca12sb = pers.tile([128, POOL12_P * PAIR], BF16, tag="ca12sb")
            acc11 = pers.tile([128, 8 * TILES], F32, tag="acc11")
            acc22 = pers.tile([128, 8 * TILES], F32, tag="acc22")
            acc12 = pers.tile([128, 16 * TILES], F32, tag="acc12")

            # input DMAs, chunked so the first window can start early
            for c0 in range(0, ANT_COLS, 4096):
                c1 = min(c0 + 4096, ANT_COLS)
                nc.sync.dma_start(anT[:, c0:c1], anT_d[:, c0:c1])
            for c0 in range(0, N, 4096):
                nc.sync.dma_start(bnT[:, c0 : c0 + 4096], bnT_d[:, c0 : c0 + 4096])

            # zero-init DRAM column accumulators so every colacc DMA can be a
            # plain accumulate (no first-touch copies; sync queue, validated
            # safe ahead of the gpsimd accumulate stream)
            zt = pers.tile([128, PAIR], BF16, tag="zt")
            nc.gpsimd.memset(zt[:], 0.0)
            for zd, z0, z1 in (
                (ca11_d, 0, CA_COLS),
                (ca22_d, 0, CA_COLS),
                (ca12_d, POOL12_P * PAIR, N),
            ):
                for c0 in range(z0, z1, PAIR):
                    c1 = min(c0 + PAIR, z1)
                    nc.sync.dma_start(zd[:, c0:c1], zt[:, 0 : c1 - c0])

            with (
                tc.tile_pool(name="mp", bufs=4, space="PSUM") as mp,
                tc.tile_pool(name="ep", bufs=1) as ep,
                tc.tile_pool(name="scr", bufs=2) as scr,
            ):
                mats = [
                    (anT, anT, acc11, acc11_d, ca11_d, band11_d, True),
                    (bnT, bnT, acc22, acc22_d, ca22_d, band22_d, True),
                    (anT, bnT, acc12, acc12_d, ca12_d, None, False),
                ]
                noff = [0, 0]  # Bresenham state: [sub-chunks seen, offloaded]
                for lhs, rhs, acc, acc_d, ca_d, band_d, sym in mats:
                    nch = 8 if sym else 16
                    for m in range(TILES):
                        lT = lhs[:, m * 128 : (m + 1) * 128]
                        base = m * 128 if sym else 0
                        E4 = None
                        for k in range(nch):
                            c0 = base + k * CHUNK
                            ps = mp.tile([128, CHUNK], F32, tag="mm")
                            for q in range(2):
                                nc.tensor.matmul(
                                    ps[:, q * 512 : (q + 1) * 512],
                                    lT,
                                    rhs[:, c0 + q * 512 : c0 + (q + 1) * 512],
                                )
                            # E tiles pack 8 sub-chunks; colacc reads 2048-wide
                            # pair slices (hw limit for accumulate DMAs)
                            if k % 8 == 0:
                                E4 = ep.tile([128, 8 * CHUNK], BF16, tag="E4", bufs=3)
                            ke = (k % 8) * CHUNK
                            E = E4[:, ke : ke + CHUNK]
                            Ei = E4[:, ke : ke + CHUNK].bitcast(I16)
                            slot = m * nch + k
                            noff[0] += 1
                            off = noff[0] * OFF_NUM // OFF_DEN > noff[1]
                            if off:
                                noff[1] += 1
                                nc.vector.tensor_scalar(
                                    Ei, ps[:], SCHRA_A, SCHRA_B, OP.mult, OP.add
                                )
                                # fast rowsum: identity tensor_scalar keeps DVE 4x
                                # mode (scalar-shaped accum doesn't break it)
                                pscr = scr.tile([128, CHUNK], BF16, tag="pscr")
                                nc.vector.tensor_scalar(
                                    pscr[:],
                                    E,
                                    1.0,
                                    None,
                                    OP.mult,
                                    OP.add,
                                    accum_out=acc[:, slot : slot + 1],
                                )
                            else:
                                nc.scalar.activation(
                                    E,
                                    ps[:],
                                    AF.Exp,
                                    scale=INV_TAU,
                                    accum_out=acc[:, slot : slot + 1],
                                )
                            if k % 2 == 0:
                                continue
                            # pair p = k//2 complete: emit column accumulation
                            p = k // 2
                            pe0 = (p % 4) * PAIR  # pair offset within E4
                            if sym:
                                # region D in [1,63]: cols [m*128+128, m*128+8192)
                                # -> ca idx [m*128, m*128+8064); DRAM was
                                # zero-initialized, so always accumulate
                                pc0 = base + p * PAIR
                                lo = max(pc0, m * 128 + 128)
                                e0 = pe0 + (lo - pc0)
                                a0 = lo - 128
                                a1 = pc0 + PAIR - 128
                                nc.gpsimd.dma_start(
                                    ca_d[:, a0:a1],
                                    E4[:, e0 : pe0 + PAIR],
                                    accum_op=OP.add,
                                )
                            elif p < POOL12_P:
                                ve = nc.vector if p < DVE12_P else nc.gpsimd
                                pc0 = p * PAIR
                                if m == 0:
                                    ve.tensor_copy(
                                        ca12sb[:, pc0 : pc0 + PAIR],
                                        E4[:, pe0 : pe0 + PAIR],
                                    )
                                else:
                                    ve.tensor_tensor(
                                        ca12sb[:, pc0 : pc0 + PAIR],
                                        E4[:, pe0 : pe0 + PAIR],
                                        ca12sb[:, pc0 : pc0 + PAIR],
                                        OP.add,
                                    )
                            else:
                                pc0 = p * PAIR
                                nc.gpsimd.dma_start(
                                    ca_d[:, pc0 : pc0 + PAIR],
                                    E4[:, pe0 : pe0 + PAIR],
                                    accum_op=OP.add,
                                )
                    if sym:
                        # band pass: distance-64 tiles (m, m+64), rowsum-only,
                        # raw exps exported; host reduces.
                        for h in range(2):
                            ps = mp.tile([128, CHUNK], F32, tag="mm")
                            for j in range(8):
                                m = h * 8 + j
                                nc.tensor.matmul(
                                    ps[:, j * 128 : (j + 1) * 128],
                                    lhs[:, m * 128 : (m + 1) * 128],
                                    rhs[:, WIN + m * 128 : WIN + (m + 1) * 128],
                                )
                            Eb = ep.tile([128, CHUNK], BF16, tag="Eb", bufs=2)
                            nc.scalar.activation(Eb[:], ps[:], AF.Exp, scale=INV_TAU)
                            nc.sync.dma_start(
                                band_d[:, h * CHUNK : (h + 1) * CHUNK], Eb[:]
                            )
                    nc.sync.dma_start(acc_d[:, :], acc[:, :])
                    if not sym:
                        for c0 in range(0, POOL12_P * PAIR, PAIR):
                            nc.sync.dma_start(
                                ca_d[:, c0 : c0 + PAIR], ca12sb[:, c0 : c0 + PAIR]
                            )

    return nc


def _get_nc():
    global _NC_CACHE
    if _NC_CACHE is None:
        _NC_CACHE = _build()
    return _NC_CACHE


def _project(z, W1, b1, W2, b2):
    u = z @ W1 + b1
    h = np.where(u > 0, u, np.expm1(np.minimum(u, 0.0))) @ W2 + b2
    n = np.sqrt(np.sum(h * h, axis=1, keepdims=True))
    return h / np.maximum(n, 1e-12)


def kernel(z1, z2, W1, b1, W2, b2):
    global LAST_RES
    bf = ml_dtypes.bfloat16
    z1 = np.asarray(z1, dtype=np.float32)
    z2 = np.asarray(z2, dtype=np.float32)
    W1 = np.asarray(W1, dtype=np.float32)
    W2 = np.asarray(W2, dtype=np.float32)
    b1 = np.asarray(b1, dtype=np.float32)
    b2 = np.asarray(b2, dtype=np.float32)

    an = _project(z1, W1, b1, W2, b2)
    bn = _project(z2, W1, b1, W2, b2)
    anT_bf = np.ascontiguousarray(an.T).astype(bf)  # [128, N]
    bnT_bf = np.ascontiguousarray(bn.T).astype(bf)

    nc = _get_nc()
    in_maps = []
    for c in range(NCORES):
        a = np.roll(anT_bf, -c * R, axis=1)
        b = np.roll(bnT_bf, -c * R, axis=1)
        in_maps.append(
            {
                "anT": np.ascontiguousarray(a[:, :ANT_COLS]),
                "bnT": np.ascontiguousarray(b),
            }
        )
    res = run_bass_kernel_spmd(nc, in_maps, list(range(NCORES)), **RUN_KWARGS)
    LAST_RES = res

    e2 = np.exp(np.float64(INV_TAU))
    den1 = np.zeros(N, np.float64)
    den2 = np.zeros(N, np.float64)
    idx_ca = None
    for c in range(NCORES):
        r = res.results[c]
        own = slice(c * R, (c + 1) * R)
        # windowed row sums: acc[p, m*nch+k] for row m*128+p
        a11 = r["acc11"].astype(np.float64).reshape(128, TILES, 8)
        a22 = r["acc22"].astype(np.float64).reshape(128, TILES, 8)
        a12 = r["acc12"].astype(np.float64).reshape(128, TILES, 16)
        rs11 = a11.sum(axis=2).T.reshape(R)  # [m,p] -> row m*128+p
        rs22 = a22.sum(axis=2).T.reshape(R)
        rs12 = a12.sum(axis=2).T.reshape(R)
        # band: E[p, m*128+j] = exp tile (m, m+64) -> row m*128+p sums over j
        b11 = r["band11"].astype(np.float64).reshape(128, TILES, 128)
        b22 = r["band22"].astype(np.float64).reshape(128, TILES, 128)
        rs11 += b11.sum(axis=2).T.reshape(R)
        rs22 += b22.sum(axis=2).T.reshape(R)
        den1[own] += rs11 + rs12
        den2[own] += rs22
        # column accumulators: partition-sum then roll to global rows
        cs11 = r["ca11"].astype(np.float64).sum(axis=0)  # local col j+128
        cs22 = r["ca22"].astype(np.float64).sum(axis=0)
        cs12 = r["ca12"].astype(np.float64).sum(axis=0)  # local col j
        if idx_ca is None:
            idx_ca = np.arange(CA_COLS)
        den1[(c * R + 128 + idx_ca) % N] += cs11
        den2[(c * R + 128 + idx_ca) % N] += cs22
        den2[(c * R + np.arange(N)) % N] += cs12
    den1 -= e2
    den2 -= e2

    lognum = 2.0 * np.sum(an.astype(np.float64) * bn.astype(np.float64), axis=1)
    loss = np.mean(0.5 * (np.log(den1) + np.log(den2)) - lognum)
    return np.array(loss, dtype=np.float32)


# revision 28
# speedup vs baseline: 1.1230x; 1.1230x over previous
"""Trainium2 Bass kernel for the HGCA contrastive loss (nn_HGCA_10857677324785).

loss = mean_i 0.5*(l1_i + l2_i),
  l1_i = log(den1_i) - log(num_i), den1_i = sum_j e^{2 an_i.an_j} + sum_j e^{2 an_i.bn_j} - e^2
  l2_i = log(den2_i) - log(num_i), den2_i = sum_j e^{2 bn_i.bn_j} + sum_j e^{2 bn_i.an_j} - e^2
where an/bn are the L2-normalized projections elu-MLP(z1/z2).

Distribution: the projections (O(N D^2), 0.5% of the FLOPs) are computed on
the host in f32 — the sharding hint's "all-gathered normalized projections"
— and handed to every core pre-transposed ([d, i] layout, bf16) and
row-rolled so each core's 2048 rows sit at local columns [0, 2048).

Each core computes its row-block of the three N x N similarity exps.  The
symmetric matrices S11 = an@an.T and S22 = bn@bn.T are only half-computed:
each 128-row tile m processes a diagonal-anchored window of 64 column tiles
(local cols [m*128, m*128+8192)) plus the distance-64 tile as a separate
"band" pass.  Row sums come from the ACT engine's fused accumulator; column
sums of the D in [1,63] part are accumulated in bf16 on the DVE and exported
raw — by symmetry they are exactly the row-sum contributions of the
uncomputed distance >= 65 tiles.  S12 is not symmetric: full rows with both
row sums (ACT accum) and bf16 column accumulation (DVE).  The host sums the
raw column accumulators over partitions, rolls them into global row space,
and assembles the scalar loss in f64 (log num_i = 2 an_i.bn_i directly).
"""

import ml_dtypes
import numpy as np

import concourse.bass as bass
import concourse.tile as tile
from concourse import mybir
from concourse.bass_utils import run_bass_kernel_spmd

N = 16384
D = 128
NCORES = 8
R = N // NCORES  # 2048 rows per core
TILES = R // 128  # 16 row tiles per core
WIN = 8192  # window: distance tiles 0..63
CHUNK = 1024  # psum/exp sub-chunk width (2-bank PSUM tiles, 4-deep rotation)
PAIR = 2048  # column-accumulate granularity (hw limit for accum DMAs)
CA_COLS = 15 * 128 + WIN - 128  # 9984: colacc for D in [1,63]
ANT_COLS = 15 * 128 + WIN + 2048  # 10240: rightmost anT column ever read
INV_TAU = 2.0  # 1/0.5
F32 = mybir.dt.float32
BF16 = mybir.dt.bfloat16
I16 = mybir.dt.int16
AF = mybir.ActivationFunctionType
OP = mybir.AluOpType

# Schraudolph fast-exp on the DVE: I = int16(A*s + B); bf16-bits(I) ~ exp(2s).
# A folds in 1/tau; B calibrated for zero mean multiplicative bias over the
# (near-uniform) mantissa phase.  Offloads ACT-engine exp work per chunk.
SCHRA_A = 2.0 * 128.0 / float(np.log(2.0))
SCHRA_B = 16250.0
OFF_NUM, OFF_DEN = 223, 512  # fraction of sub-chunks exp'd on DVE (Bresenham)
DVE12_P = 1  # S12 column pairs p < this accumulate on DVE (SBUF)
POOL12_P = 2  # S12 pairs p in [DVE12_P, this) accumulate on gpsimd (SBUF)
# S12 pairs p >= POOL12_P go per-pair SWDGE-accumulate to DRAM

# This walrus build supports at most 2 sync waits per instruction; Tile's sem
# assignment freely emits 3-11. Post-pass: hoist excess waits onto injected
# same-engine EventSemaphore fillers (engine queues are FIFO, so waits on an
# earlier filler happen-before the original instruction executes).

_MAX_WAITS = 1


def _split_waits(nc):
    for fn in nc.m.functions:
        for bb in fn.blocks:
            insts = list(bb.instructions)
            out = []
            changed = False
            for inst in insts:
                si = inst.sync_info
                w = list(si.on_wait) if si and si.on_wait else []
                if len(w) > _MAX_WAITS:
                    changed = True
                    extra, keep = w[:-_MAX_WAITS], w[-_MAX_WAITS:]
                    for i in range(0, len(extra), _MAX_WAITS):
                        f = mybir.InstEventSemaphore(
                            name=f"{inst.name}_wsplit{i}",
                            engine=inst.engine,
                            ins=[],
                            outs=[],
                            sync_info=mybir.SyncInfo(
                                on_wait=extra[i : i + _MAX_WAITS], on_update=[]
                            ),
                        )
                        out.append(f)
                    inst.sync_info = mybir.SyncInfo(
                        on_wait=keep,
                        on_update=list(si.on_update) if si.on_update else [],
                    )
                out.append(inst)
            if changed:
                bb.instructions = out


def _patched_drain_and_barrier(self, tick_clock, wait_clock):
    from concourse.vector_clock import ScopedClock

    nc = self.nc
    drain_inst = nc.sync.drain()
    wait_clock.add_sem_waits(
        drain_inst.ins, ScopedClock({None: tick_clock.global_clock})
    )
    nc.all_engine_barrier()
    assert self.sems is not None
    popped = nc._tile_sem_poison_stack.pop()
    assert popped is self._sem_poison
    nc.clear_and_free_semaphores(list(self.sems.allocated().values()))
    nc.all_engine_barrier()
    _split_waits(nc)


tile.TileContext._drain_and_barrier = _patched_drain_and_barrier

_NC_CACHE = None
RUN_KWARGS: dict = {}
LAST_RES = None


def _build():
    nc = bass.Bass("TRN2", target_bir_lowering=False, debug=False)

    anT_d = nc.dram_tensor("anT", [128, ANT_COLS], BF16, kind="ExternalInput").ap()
    bnT_d = nc.dram_tensor("bnT", [128, N], BF16, kind="ExternalInput").ap()

    acc11_d = nc.dram_tensor("acc11", [128, 8 * TILES], F32, kind="ExternalOutput").ap()
    acc22_d = nc.dram_tensor("acc22", [128, 8 * TILES], F32, kind="ExternalOutput").ap()
    acc12_d = nc.dram_tensor("acc12", [128, 16 * TILES], F32, kind="ExternalOutput").ap()
    band11_d = nc.dram_tensor("band11", [128, R], BF16, kind="ExternalOutput").ap()
    band22_d = nc.dram_tensor("band22", [128, R], BF16, kind="ExternalOutput").ap()
    ca11_d = nc.dram_tensor("ca11", [128, CA_COLS], BF16, kind="ExternalOutput").ap()
    ca22_d = nc.dram_tensor("ca22", [128, CA_COLS], BF16, kind="ExternalOutput").ap()
    ca12_d = nc.dram_tensor("ca12", [128, N], BF16, kind="ExternalOutput").ap()

    with tile.TileContext(nc) as tc:
        with tc.tile_pool(name="pers", bufs=1) as pers:
            anT = pers.tile([128, ANT_COLS], BF16, tag="anT")
            bnT = pers.tile([128, N], BF16, tag="bnT")
            ca12sb = pers.tile([128, POOL12_P * PAIR], BF16, tag="ca12sb")
            acc11 = pers.tile([128, 8 * TILES], F32, tag="acc11")
            acc22 = pers.tile([128, 8 * TILES], F32, tag="acc22")
            acc12 = pers.tile([128, 16 * TILES], F32, tag="acc12")

            # input DMAs, chunked so the first window can start early
            for c0 in range(0, ANT_COLS, 4096):
                c1 = min(c0 + 4096, ANT_COLS)
                nc.sync.dma_start(anT[:, c0:c1], anT_d[:, c0:c1])
            for c0 in range(0, N, 4096):
                nc.sync.dma_start(bnT[:, c0 : c0 + 4096], bnT_d[:, c0 : c0 + 4096])

            with (
                tc.tile_pool(name="mp", bufs=4, space="PSUM") as mp,
                tc.tile_pool(name="ep", bufs=1) as ep,
                tc.tile_pool(name="scr", bufs=2) as scr,
            ):
                mats = [
                    (anT, anT, acc11, acc11_d, ca11_d, band11_d, True),
                    (bnT, bnT, acc22, acc22_d, ca22_d, band22_d, True),
                    (anT, bnT, acc12, acc12_d, ca12_d, None, False),
                ]
                noff = [0, 0]  # Bresenham state: [sub-chunks seen, offloaded]
                for lhs, rhs, acc, acc_d, ca_d, band_d, sym in mats:
                    nch = 8 if sym else 16
                    for m in range(TILES):
                        lT = lhs[:, m * 128 : (m + 1) * 128]
                        base = m * 128 if sym else 0
                        E4 = None
                        for k in range(nch):
                            c0 = base + k * CHUNK
                            ps = mp.tile([128, CHUNK], F32, tag="mm")
                            for q in range(2):
                                nc.tensor.matmul(
                                    ps[:, q * 512 : (q + 1) * 512],
                                    lT,
                                    rhs[:, c0 + q * 512 : c0 + (q + 1) * 512],
                                )
                            # E tiles pack 8 sub-chunks; colacc reads 2048-wide
                            # pair slices (hw limit for accumulate DMAs)
                            if k % 8 == 0:
                                E4 = ep.tile([128, 8 * CHUNK], BF16, tag="E4", bufs=3)
                            ke = (k % 8) * CHUNK
                            E = E4[:, ke : ke + CHUNK]
                            Ei = E4[:, ke : ke + CHUNK].bitcast(I16)
                            slot = m * nch + k
                            noff[0] += 1
                            off = noff[0] * OFF_NUM // OFF_DEN > noff[1]
                            if off:
                                noff[1] += 1
                                nc.vector.tensor_scalar(
                                    Ei, ps[:], SCHRA_A, SCHRA_B, OP.mult, OP.add
                                )
                                # fast rowsum: identity tensor_scalar keeps DVE 4x
                                # mode (scalar-shaped accum doesn't break it)
                                pscr = scr.tile([128, CHUNK], BF16, tag="pscr")
                                nc.vector.tensor_scalar(
                                    pscr[:],
                                    E,
                                    1.0,
                                    None,
                                    OP.mult,
                                    OP.add,
                                    accum_out=acc[:, slot : slot + 1],
                                )
                            else:
                                nc.scalar.activation(
                                    E,
                                    ps[:],
                                    AF.Exp,
                                    scale=INV_TAU,
                                    accum_out=acc[:, slot : slot + 1],
                                )
                            if k % 2 == 0:
                                continue
                            # pair p = k//2 complete: emit column accumulation
                            p = k // 2
                            pe0 = (p % 4) * PAIR  # pair offset within E4
                            if sym:
                                # region D in [1,63]: cols [m*128+128, m*128+8192)
                                # -> ca idx [m*128, m*128+8064); cols >= prev
                                # tile end (ca idx >= m*128+7936) first-touched
                                pc0 = base + p * PAIR
                                lo = max(pc0, m * 128 + 128)
                                hi = pc0 + PAIR
                                e0 = pe0 + (lo - pc0)
                                a0 = lo - 128
                                a1 = hi - 128
                                new0 = 128 if m == 0 else m * 128 + 8064
                                if lo >= new0:
                                    nc.gpsimd.dma_start(
                                        ca_d[:, a0:a1], E4[:, e0 : pe0 + PAIR]
                                    )
                                elif hi <= new0:
                                    nc.gpsimd.dma_start(
                                        ca_d[:, a0:a1],
                                        E4[:, e0 : pe0 + PAIR],
                                        accum_op=OP.add,
                                    )
                                else:
                                    sp = pe0 + (new0 - pc0)
                                    nc.gpsimd.dma_start(
                                        ca_d[:, a0 : new0 - 128],
                                        E4[:, e0:sp],
                                        accum_op=OP.add,
                                    )
                                    nc.gpsimd.dma_start(
                                        ca_d[:, new0 - 128 : a1],
                                        E4[:, sp : pe0 + PAIR],
                                    )
                            elif p < POOL12_P:
                                ve = nc.vector if p < DVE12_P else nc.gpsimd
                                pc0 = p * PAIR
                                if m == 0:
                                    ve.tensor_copy(
                                        ca12sb[:, pc0 : pc0 + PAIR],
                                        E4[:, pe0 : pe0 + PAIR],
                                    )
                                else:
                                    ve.tensor_tensor(
                                        ca12sb[:, pc0 : pc0 + PAIR],
                                        E4[:, pe0 : pe0 + PAIR],
                                        ca12sb[:, pc0 : pc0 + PAIR],
                                        OP.add,
                                    )
                            else:
                                pc0 = p * PAIR
                                if m == 0:
                                    nc.gpsimd.dma_start(
                                        ca_d[:, pc0 : pc0 + PAIR],
                                        E4[:, pe0 : pe0 + PAIR],
                                    )
                                else:
                                    nc.gpsimd.dma_start(
                                        ca_d[:, pc0 : pc0 + PAIR],
                                        E4[:, pe0 : pe0 + PAIR],
                                        accum_op=OP.add,
                                    )
                    if sym:
                        # band pass: distance-64 tiles (m, m+64), rowsum-only,
                        # raw exps exported; host reduces.
                        for h in range(2):
                            ps = mp.tile([128, CHUNK], F32, tag="mm")
                            for j in range(8):
                                m = h * 8 + j
                                nc.tensor.matmul(
                                    ps[:, j * 128 : (j + 1) * 128],
                                    lhs[:, m * 128 : (m + 1) * 128],
                                    rhs[:, WIN + m * 128 : WIN + (m + 1) * 128],
                                )
                            Eb = ep.tile([128, CHUNK], BF16, tag="Eb", bufs=2)
                            nc.scalar.activation(Eb[:], ps[:], AF.Exp, scale=INV_TAU)
                            nc.sync.dma_start(
                                band_d[:, h * CHUNK : (h + 1) * CHUNK], Eb[:]
                            )
                    nc.sync.dma_start(acc_d[:, :], acc[:, :])
                    if not sym:
                        for c0 in range(0, POOL12_P * PAIR, PAIR):
                            nc.sync.dma_start(
                                ca_d[:, c0 : c0 + PAIR], ca12sb[:, c0 : c0 + PAIR]
                            )

    return nc


def _get_nc():
    global _NC_CACHE
    if _NC_CACHE is None:
        _NC_CACHE = _build()
    return _NC_CACHE


def _project(z, W1, b1, W2, b2):
    u = z @ W1 + b1
    h = np.where(u > 0, u, np.expm1(np.minimum(u, 0.0))) @ W2 + b2
    n = np.sqrt(np.sum(h * h, axis=1, keepdims=True))
    return h / np.maximum(n, 1e-12)


def kernel(z1, z2, W1, b1, W2, b2):
    global LAST_RES
    bf = ml_dtypes.bfloat16
    z1 = np.asarray(z1, dtype=np.float32)
    z2 = np.asarray(z2, dtype=np.float32)
    W1 = np.asarray(W1, dtype=np.float32)
    W2 = np.asarray(W2, dtype=np.float32)
    b1 = np.asarray(b1, dtype=np.float32)
    b2 = np.asarray(b2, dtype=np.float32)

    an = _project(z1, W1, b1, W2, b2)
    bn = _project(z2, W1, b1, W2, b2)
    anT_bf = np.ascontiguousarray(an.T).astype(bf)  # [128, N]
    bnT_bf = np.ascontiguousarray(bn.T).astype(bf)

    nc = _get_nc()
    in_maps = []
    for c in range(NCORES):
        a = np.roll(anT_bf, -c * R, axis=1)
        b = np.roll(bnT_bf, -c * R, axis=1)
        in_maps.append(
            {
                "anT": np.ascontiguousarray(a[:, :ANT_COLS]),
                "bnT": np.ascontiguousarray(b),
            }
        )
    res = run_bass_kernel_spmd(nc, in_maps, list(range(NCORES)), **RUN_KWARGS)
    LAST_RES = res

    e2 = np.exp(np.float64(INV_TAU))
    den1 = np.zeros(N, np.float64)
    den2 = np.zeros(N, np.float64)
    idx_ca = None
    for c in range(NCORES):
        r = res.results[c]
        own = slice(c * R, (c + 1) * R)
        # windowed row sums: acc[p, m*nch+k] for row m*128+p
        a11 = r["acc11"].astype(np.float64).reshape(128, TILES, 8)
        a22 = r["acc22"].astype(np.float64).reshape(128, TILES, 8)
        a12 = r["acc12"].astype(np.float64).reshape(128, TILES, 16)
        rs11 = a11.sum(axis=2).T.reshape(R)  # [m,p] -> row m*128+p
        rs22 = a22.sum(axis=2).T.reshape(R)
        rs12 = a12.sum(axis=2).T.reshape(R)
        # band: E[p, m*128+j] = exp tile (m, m+64) -> row m*128+p sums over j
        b11 = r["band11"].astype(np.float64).reshape(128, TILES, 128)
        b22 = r["band22"].astype(np.float64).reshape(128, TILES, 128)
        rs11 += b11.sum(axis=2).T.reshape(R)
        rs22 += b22.sum(axis=2).T.reshape(R)
        den1[own] += rs11 + rs12
        den2[own] += rs22
        # column accumulators: partition-sum then roll to global rows
        cs11 = r["ca11"].astype(np.float64).sum(axis=0)  # local col j+128
        cs22 = r["ca22"].astype(np.float64).sum(axis=0)
        cs12 = r["ca12"].astype(np.float64).sum(axis=0)  # local col j
        if idx_ca is None:
            idx_ca = np.arange(CA_COLS)
        den1[(c * R + 128 + idx_ca) % N] += cs11
        den2[(c * R + 128 + idx_ca) % N] += cs22
        den2[(c * R + np.arange(N)) % N] += cs12
    den1 -= e2
    den2 -= e2

    lognum = 2.0 * np.sum(an.astype(np.float64) * bn.astype(np.float64), axis=1)
    loss = np.mean(0.5 * (np.log(den1) + np.log(den2)) - lognum)
    return np.array(loss, dtype=np.float32)


# revision 29
# speedup vs baseline: 1.1234x; 1.0004x over previous
"""Trainium2 Bass kernel for the HGCA contrastive loss (nn_HGCA_10857677324785).

loss = mean_i 0.5*(l1_i + l2_i),
  l1_i = log(den1_i) - log(num_i), den1_i = sum_j e^{2 an_i.an_j} + sum_j e^{2 an_i.bn_j} - e^2
  l2_i = log(den2_i) - log(num_i), den2_i = sum_j e^{2 bn_i.bn_j} + sum_j e^{2 bn_i.an_j} - e^2
where an/bn are the L2-normalized projections elu-MLP(z1/z2).

Distribution: the projections (O(N D^2), 0.5% of the FLOPs) are computed on
the host in f32 — the sharding hint's "all-gathered normalized projections"
— and handed to every core pre-transposed ([d, i] layout, bf16) and
row-rolled so each core's 2048 rows sit at local columns [0, 2048).

Each core computes its row-block of the three N x N similarity exps.  The
symmetric matrices S11 = an@an.T and S22 = bn@bn.T are only half-computed:
each 128-row tile m processes a diagonal-anchored window of 64 column tiles
(local cols [m*128, m*128+8192)) plus the distance-64 tile as a separate
"band" pass.  Row sums come from the ACT engine's fused accumulator; column
sums of the D in [1,63] part are accumulated in bf16 on the DVE and exported
raw — by symmetry they are exactly the row-sum contributions of the
uncomputed distance >= 65 tiles.  S12 is not symmetric: full rows with both
row sums (ACT accum) and bf16 column accumulation (DVE).  The host sums the
raw column accumulators over partitions, rolls them into global row space,
and assembles the scalar loss in f64 (log num_i = 2 an_i.bn_i directly).
"""

import ml_dtypes
import numpy as np

import concourse.bass as bass
import concourse.tile as tile
from concourse import mybir
from concourse.bass_utils import run_bass_kernel_spmd

N = 16384
D = 128
NCORES = 8
R = N // NCORES  # 2048 rows per core
TILES = R // 128  # 16 row tiles per core
WIN = 8192  # window: distance tiles 0..63
CHUNK = 1024  # psum/exp sub-chunk width (2-bank PSUM tiles, 4-deep rotation)
PAIR = 2048  # column-accumulate granularity (hw limit for accum DMAs)
CA_COLS = 15 * 128 + WIN - 128  # 9984: colacc for D in [1,63]
ANT_COLS = 15 * 128 + WIN + 2048  # 10240: rightmost anT column ever read
INV_TAU = 2.0  # 1/0.5
F32 = mybir.dt.float32
BF16 = mybir.dt.bfloat16
I16 = mybir.dt.int16
AF = mybir.ActivationFunctionType
OP = mybir.AluOpType

# Schraudolph fast-exp on the DVE: I = int16(A*s + B); bf16-bits(I) ~ exp(2s).
# A folds in 1/tau; B calibrated for zero mean multiplicative bias over the
# (near-uniform) mantissa phase.  Offloads ACT-engine exp work per chunk.
SCHRA_A = 2.0 * 128.0 / float(np.log(2.0))
SCHRA_B = 16250.0
OFF_NUM, OFF_DEN = 223, 512  # fraction of sub-chunks exp'd on DVE (Bresenham)
SB12_P0 = 6  # S12 pairs p >= this accumulate in SBUF (gpsimd p=6, DVE p=7);
# pairs p < 6 go per-pair SWDGE-accumulate to DRAM.  SBUF pairs sit last so
# the kernel tail is a short HWDGE export, not a chained SWDGE accumulate.

# This walrus build supports at most 2 sync waits per instruction; Tile's sem
# assignment freely emits 3-11. Post-pass: hoist excess waits onto injected
# same-engine EventSemaphore fillers (engine queues are FIFO, so waits on an
# earlier filler happen-before the original instruction executes).

_MAX_WAITS = 1


def _split_waits(nc):
    for fn in nc.m.functions:
        for bb in fn.blocks:
            insts = list(bb.instructions)
            out = []
            changed = False
            for inst in insts:
                si = inst.sync_info
                w = list(si.on_wait) if si and si.on_wait else []
                if len(w) > _MAX_WAITS:
                    changed = True
                    extra, keep = w[:-_MAX_WAITS], w[-_MAX_WAITS:]
                    for i in range(0, len(extra), _MAX_WAITS):
                        f = mybir.InstEventSemaphore(
                            name=f"{inst.name}_wsplit{i}",
                            engine=inst.engine,
                            ins=[],
                            outs=[],
                            sync_info=mybir.SyncInfo(
                                on_wait=extra[i : i + _MAX_WAITS], on_update=[]
                            ),
                        )
                        out.append(f)
                    inst.sync_info = mybir.SyncInfo(
                        on_wait=keep,
                        on_update=list(si.on_update) if si.on_update else [],
                    )
                out.append(inst)
            if changed:
                bb.instructions = out


def _patched_drain_and_barrier(self, tick_clock, wait_clock):
    from concourse.vector_clock import ScopedClock

    nc = self.nc
    drain_inst = nc.sync.drain()
    wait_clock.add_sem_waits(
        drain_inst.ins, ScopedClock({None: tick_clock.global_clock})
    )
    nc.all_engine_barrier()
    assert self.sems is not None
    popped = nc._tile_sem_poison_stack.pop()
    assert popped is self._sem_poison
    nc.clear_and_free_semaphores(list(self.sems.allocated().values()))
    nc.all_engine_barrier()
    _split_waits(nc)


tile.TileContext._drain_and_barrier = _patched_drain_and_barrier

_NC_CACHE = None
RUN_KWARGS: dict = {}
LAST_RES = None


def _build():
    nc = bass.Bass("TRN2", target_bir_lowering=False, debug=False)

    anT_d = nc.dram_tensor("anT", [128, ANT_COLS], BF16, kind="ExternalInput").ap()
    bnT_d = nc.dram_tensor("bnT", [128, N], BF16, kind="ExternalInput").ap()

    acc11_d = nc.dram_tensor("acc11", [128, 8 * TILES], F32, kind="ExternalOutput").ap()
    acc22_d = nc.dram_tensor("acc22", [128, 8 * TILES], F32, kind="ExternalOutput").ap()
    acc12_d = nc.dram_tensor("acc12", [128, 16 * TILES], F32, kind="ExternalOutput").ap()
    band11_d = nc.dram_tensor("band11", [128, R], BF16, kind="ExternalOutput").ap()
    band22_d = nc.dram_tensor("band22", [128, R], BF16, kind="ExternalOutput").ap()
    ca11_d = nc.dram_tensor("ca11", [128, CA_COLS], BF16, kind="ExternalOutput").ap()
    ca22_d = nc.dram_tensor("ca22", [128, CA_COLS], BF16, kind="ExternalOutput").ap()
    ca12_d = nc.dram_tensor("ca12", [128, N], BF16, kind="ExternalOutput").ap()

    with tile.TileContext(nc) as tc:
        with tc.tile_pool(name="pers", bufs=1) as pers:
            anT = pers.tile([128, ANT_COLS], BF16, tag="anT")
            bnT = pers.tile([128, N], BF16, tag="bnT")
            ca12sb = pers.tile([128, (8 - SB12_P0) * PAIR], BF16, tag="ca12sb")
            acc11 = pers.tile([128, 8 * TILES], F32, tag="acc11")
            acc22 = pers.tile([128, 8 * TILES], F32, tag="acc22")
            acc12 = pers.tile([128, 16 * TILES], F32, tag="acc12")

            # input DMAs, chunked so the first window can start early
            for c0, c1 in [(0, 2048), (2048, 4096)] + [
                (x, min(x + 4096, ANT_COLS)) for x in range(4096, ANT_COLS, 4096)
            ]:
                nc.sync.dma_start(anT[:, c0:c1], anT_d[:, c0:c1])
            for c0 in range(0, N, 4096):
                nc.sync.dma_start(bnT[:, c0 : c0 + 4096], bnT_d[:, c0 : c0 + 4096])

            with (
                tc.tile_pool(name="mp", bufs=4, space="PSUM") as mp,
                tc.tile_pool(name="ep", bufs=1) as ep,
                tc.tile_pool(name="scr", bufs=2) as scr,
            ):
                mats = [
                    (anT, anT, acc11, acc11_d, ca11_d, band11_d, True),
                    (bnT, bnT, acc22, acc22_d, ca22_d, band22_d, True),
                    (anT, bnT, acc12, acc12_d, ca12_d, None, False),
                ]
                noff = [0, 0]  # Bresenham state: [sub-chunks seen, offloaded]
                for lhs, rhs, acc, acc_d, ca_d, band_d, sym in mats:
                    nch = 8 if sym else 16
                    for m in range(TILES):
                        lT = lhs[:, m * 128 : (m + 1) * 128]
                        base = m * 128 if sym else 0
                        E4 = None
                        for k in range(nch):
                            c0 = base + k * CHUNK
                            ps = mp.tile([128, CHUNK], F32, tag="mm")
                            for q in range(2):
                                nc.tensor.matmul(
                                    ps[:, q * 512 : (q + 1) * 512],
                                    lT,
                                    rhs[:, c0 + q * 512 : c0 + (q + 1) * 512],
                                )
                            # E tiles pack 8 sub-chunks; colacc reads 2048-wide
                            # pair slices (hw limit for accumulate DMAs)
                            if k % 8 == 0:
                                E4 = ep.tile([128, 8 * CHUNK], BF16, tag="E4", bufs=3)
                            ke = (k % 8) * CHUNK
                            E = E4[:, ke : ke + CHUNK]
                            Ei = E4[:, ke : ke + CHUNK].bitcast(I16)
                            slot = m * nch + k
                            noff[0] += 1
                            off = noff[0] * OFF_NUM // OFF_DEN > noff[1]
                            if off:
                                noff[1] += 1
                                nc.vector.tensor_scalar(
                                    Ei, ps[:], SCHRA_A, SCHRA_B, OP.mult, OP.add
                                )
                                # fast rowsum: identity tensor_scalar keeps DVE 4x
                                # mode (scalar-shaped accum doesn't break it)
                                pscr = scr.tile([128, CHUNK], BF16, tag="pscr")
                                nc.vector.tensor_scalar(
                                    pscr[:],
                                    E,
                                    1.0,
                                    None,
                                    OP.mult,
                                    OP.add,
                                    accum_out=acc[:, slot : slot + 1],
                                )
                            else:
                                nc.scalar.activation(
                                    E,
                                    ps[:],
                                    AF.Exp,
                                    scale=INV_TAU,
                                    accum_out=acc[:, slot : slot + 1],
                                )
                            if k % 2 == 0:
                                continue
                            # pair p = k//2 complete: emit column accumulation
                            p = k // 2
                            pe0 = (p % 4) * PAIR  # pair offset within E4
                            if sym:
                                # region D in [1,63]: cols [m*128+128, m*128+8192)
                                # -> ca idx [m*128, m*128+8064); cols >= prev
                                # tile end (ca idx >= m*128+7936) first-touched
                                pc0 = base + p * PAIR
                                lo = max(pc0, m * 128 + 128)
                                hi = pc0 + PAIR
                                e0 = pe0 + (lo - pc0)
                                a0 = lo - 128
                                a1 = hi - 128
                                new0 = 128 if m == 0 else m * 128 + 8064
                                if lo >= new0:
                                    nc.gpsimd.dma_start(
                                        ca_d[:, a0:a1], E4[:, e0 : pe0 + PAIR]
                                    )
                                elif hi <= new0:
                                    nc.gpsimd.dma_start(
                                        ca_d[:, a0:a1],
                                        E4[:, e0 : pe0 + PAIR],
                                        accum_op=OP.add,
                                    )
                                else:
                                    sp = pe0 + (new0 - pc0)
                                    nc.gpsimd.dma_start(
                                        ca_d[:, a0 : new0 - 128],
                                        E4[:, e0:sp],
                                        accum_op=OP.add,
                                    )
                                    nc.gpsimd.dma_start(
                                        ca_d[:, new0 - 128 : a1],
                                        E4[:, sp : pe0 + PAIR],
                                    )
                            elif p >= SB12_P0:
                                ve = nc.gpsimd if p == SB12_P0 else nc.vector
                                pc0 = (p - SB12_P0) * PAIR
                                if m == 0:
                                    ve.tensor_copy(
                                        ca12sb[:, pc0 : pc0 + PAIR],
                                        E4[:, pe0 : pe0 + PAIR],
                                    )
                                else:
                                    ve.tensor_tensor(
                                        ca12sb[:, pc0 : pc0 + PAIR],
                                        E4[:, pe0 : pe0 + PAIR],
                                        ca12sb[:, pc0 : pc0 + PAIR],
                                        OP.add,
                                    )
                            else:
                                pc0 = p * PAIR
                                if m == 0:
                                    nc.gpsimd.dma_start(
                                        ca_d[:, pc0 : pc0 + PAIR],
                                        E4[:, pe0 : pe0 + PAIR],
                                    )
                                else:
                                    nc.gpsimd.dma_start(
                                        ca_d[:, pc0 : pc0 + PAIR],
                                        E4[:, pe0 : pe0 + PAIR],
                                        accum_op=OP.add,
                                    )
                    if sym:
                        # band pass: distance-64 tiles (m, m+64), rowsum-only,
                        # raw exps exported; host reduces.
                        for h in range(2):
                            ps = mp.tile([128, CHUNK], F32, tag="mm")
                            for j in range(8):
                                m = h * 8 + j
                                nc.tensor.matmul(
                                    ps[:, j * 128 : (j + 1) * 128],
                                    lhs[:, m * 128 : (m + 1) * 128],
                                    rhs[:, WIN + m * 128 : WIN + (m + 1) * 128],
                                )
                            Eb = ep.tile([128, CHUNK], BF16, tag="Eb", bufs=2)
                            nc.scalar.activation(Eb[:], ps[:], AF.Exp, scale=INV_TAU)
                            nc.sync.dma_start(
                                band_d[:, h * CHUNK : (h + 1) * CHUNK], Eb[:]
                            )
                    nc.sync.dma_start(acc_d[:, :], acc[:, :])
                    if not sym:
                        for i in range(8 - SB12_P0):
                            c0 = (SB12_P0 + i) * PAIR
                            nc.sync.dma_start(
                                ca_d[:, c0 : c0 + PAIR],
                                ca12sb[:, i * PAIR : (i + 1) * PAIR],
                            )

    return nc


def _get_nc():
    global _NC_CACHE
    if _NC_CACHE is None:
        _NC_CACHE = _build()
    return _NC_CACHE


def _project(z, W1, b1, W2, b2):
    u = z @ W1 + b1
    h = np.where(u > 0, u, np.expm1(np.minimum(u, 0.0))) @ W2 + b2
    n = np.sqrt(np.sum(h * h, axis=1, keepdims=True))
    return h / np.maximum(n, 1e-12)


def kernel(z1, z2, W1, b1, W2, b2):
    global LAST_RES
    bf = ml_dtypes.bfloat16
    z1 = np.asarray(z1, dtype=np.float32)
    z2 = np.asarray(z2, dtype=np.float32)
    W1 = np.asarray(W1, dtype=np.float32)
    W2 = np.asarray(W2, dtype=np.float32)
    b1 = np.asarray(b1, dtype=np.float32)
    b2 = np.asarray(b2, dtype=np.float32)

    an = _project(z1, W1, b1, W2, b2)
    bn = _project(z2, W1, b1, W2, b2)
    anT_bf = np.ascontiguousarray(an.T).astype(bf)  # [128, N]
    bnT_bf = np.ascontiguousarray(bn.T).astype(bf)

    nc = _get_nc()
    in_maps = []
    for c in range(NCORES):
        a = np.roll(anT_bf, -c * R, axis=1)
        b = np.roll(bnT_bf, -c * R, axis=1)
        in_maps.append(
            {
                "anT": np.ascontiguousarray(a[:, :ANT_COLS]),
                "bnT": np.ascontiguousarray(b),
            }
        )
    res = run_bass_kernel_spmd(nc, in_maps, list(range(NCORES)), **RUN_KWARGS)
    LAST_RES = res

    e2 = np.exp(np.float64(INV_TAU))
    den1 = np.zeros(N, np.float64)
    den2 = np.zeros(N, np.float64)
    idx_ca = None
    for c in range(NCORES):
        r = res.results[c]
        own = slice(c * R, (c + 1) * R)
        # windowed row sums: acc[p, m*nch+k] for row m*128+p
        a11 = r["acc11"].astype(np.float64).reshape(128, TILES, 8)
        a22 = r["acc22"].astype(np.float64).reshape(128, TILES, 8)
        a12 = r["acc12"].astype(np.float64).reshape(128, TILES, 16)
        rs11 = a11.sum(axis=2).T.reshape(R)  # [m,p] -> row m*128+p
        rs22 = a22.sum(axis=2).T.reshape(R)
        rs12 = a12.sum(axis=2).T.reshape(R)
        # band: E[p, m*128+j] = exp tile (m, m+64) -> row m*128+p sums over j
        b11 = r["band11"].astype(np.float64).reshape(128, TILES, 128)
        b22 = r["band22"].astype(np.float64).reshape(128, TILES, 128)
        rs11 += b11.sum(axis=2).T.reshape(R)
        rs22 += b22.sum(axis=2).T.reshape(R)
        den1[own] += rs11 + rs12
        den2[own] += rs22
        # column accumulators: partition-sum then roll to global rows
        cs11 = r["ca11"].astype(np.float64).sum(axis=0)  # local col j+128
        cs22 = r["ca22"].astype(np.float64).sum(axis=0)
        cs12 = r["ca12"].astype(np.float64).sum(axis=0)  # local col j
        if idx_ca is None:
            idx_ca = np.arange(CA_COLS)
        den1[(c * R + 128 + idx_ca) % N] += cs11
        den2[(c * R + 128 + idx_ca) % N] += cs22
        den2[(c * R + np.arange(N)) % N] += cs12
    den1 -= e2
    den2 -= e2

    lognum = 2.0 * np.sum(an.astype(np.float64) * bn.astype(np.float64), axis=1)
    loss = np.mean(0.5 * (np.log(den1) + np.log(den2)) - lognum)
    return np.array(loss, dtype=np.float32)


# revision 30
# speedup vs baseline: 1.1444x; 1.0187x over previous
"""Trainium2 Bass kernel for the HGCA contrastive loss (nn_HGCA_10857677324785).

loss = mean_i 0.5*(l1_i + l2_i),
  l1_i = log(den1_i) - log(num_i), den1_i = sum_j e^{2 an_i.an_j} + sum_j e^{2 an_i.bn_j} - e^2
  l2_i = log(den2_i) - log(num_i), den2_i = sum_j e^{2 bn_i.bn_j} + sum_j e^{2 bn_i.an_j} - e^2
where an/bn are the L2-normalized projections elu-MLP(z1/z2).

Distribution: the projections (O(N D^2), 0.5% of the FLOPs) are computed on
the host in f32 — the sharding hint's "all-gathered normalized projections"
— and handed to every core pre-transposed ([d, i] layout, bf16) and
row-rolled so each core's 2048 rows sit at local columns [0, 2048).

Each core computes its row-block of the three N x N similarity exps.  The
symmetric matrices S11 = an@an.T and S22 = bn@bn.T are only half-computed:
each 128-row tile m processes a diagonal-anchored window of 64 column tiles
(local cols [m*128, m*128+8192)) plus the distance-64 tile as a separate
"band" pass.  Row sums come from the ACT engine's fused accumulator; column
sums of the D in [1,63] part are accumulated in bf16 on the DVE and exported
raw — by symmetry they are exactly the row-sum contributions of the
uncomputed distance >= 65 tiles.  S12 is not symmetric: full rows with both
row sums (ACT accum) and bf16 column accumulation (DVE).  The host sums the
raw column accumulators over partitions, rolls them into global row space,
and assembles the scalar loss in f64 (log num_i = 2 an_i.bn_i directly).
"""

import ml_dtypes
import numpy as np

import concourse.bass as bass
import concourse.tile as tile
from concourse import mybir
from concourse.bass_utils import run_bass_kernel_spmd

N = 16384
D = 128
NCORES = 8
R = N // NCORES  # 2048 rows per core
TILES = R // 128  # 16 row tiles per core
WIN = 8192  # window: distance tiles 0..63
CHUNK = 1024  # psum/exp sub-chunk width (2-bank PSUM tiles, 4-deep rotation)
PAIR = 2048  # column-accumulate granularity (hw limit for accum DMAs)
CA_COLS = 15 * 128 + WIN - 128  # 9984: colacc for D in [1,63]
ANT_COLS = 15 * 128 + WIN + 2048  # 10240: rightmost anT column ever read
INV_TAU = 2.0  # 1/0.5
F32 = mybir.dt.float32
BF16 = mybir.dt.bfloat16
I16 = mybir.dt.int16
AF = mybir.ActivationFunctionType
OP = mybir.AluOpType

# Schraudolph fast-exp on the DVE: I = int16(A*s + B); bf16-bits(I) ~ exp(2s).
# A folds in 1/tau; B calibrated for zero mean multiplicative bias over the
# (near-uniform) mantissa phase.  Offloads ACT-engine exp work per chunk.
SCHRA_A = 2.0 * 128.0 / float(np.log(2.0))
SCHRA_B = 16250.0
OFF_NUM, OFF_DEN = 223, 512  # fraction of sub-chunks exp'd on DVE (Bresenham)
SB12_P0 = 6  # S12 pairs p >= this accumulate in SBUF (gpsimd p=6, DVE p=7);
# pairs p < 6 go per-pair SWDGE-accumulate to DRAM.  SBUF pairs sit last so
# the kernel tail is a short HWDGE export, not a chained SWDGE accumulate.

# This walrus build supports at most 2 sync waits per instruction; Tile's sem
# assignment freely emits 3-11. Post-pass: hoist excess waits onto injected
# same-engine EventSemaphore fillers (engine queues are FIFO, so waits on an
# earlier filler happen-before the original instruction executes).

_MAX_WAITS = 1


def _split_waits(nc):
    for fn in nc.m.functions:
        for bb in fn.blocks:
            insts = list(bb.instructions)
            out = []
            changed = False
            for inst in insts:
                si = inst.sync_info
                w = list(si.on_wait) if si and si.on_wait else []
                if len(w) > _MAX_WAITS:
                    changed = True
                    extra, keep = w[:-_MAX_WAITS], w[-_MAX_WAITS:]
                    for i in range(0, len(extra), _MAX_WAITS):
                        f = mybir.InstEventSemaphore(
                            name=f"{inst.name}_wsplit{i}",
                            engine=inst.engine,
                            ins=[],
                            outs=[],
                            sync_info=mybir.SyncInfo(
                                on_wait=extra[i : i + _MAX_WAITS], on_update=[]
                            ),
                        )
                        out.append(f)
                    inst.sync_info = mybir.SyncInfo(
                        on_wait=keep,
                        on_update=list(si.on_update) if si.on_update else [],
                    )
                out.append(inst)
            if changed:
                bb.instructions = out


def _patched_drain_and_barrier(self, tick_clock, wait_clock):
    from concourse.vector_clock import ScopedClock

    nc = self.nc
    drain_inst = nc.sync.drain()
    wait_clock.add_sem_waits(
        drain_inst.ins, ScopedClock({None: tick_clock.global_clock})
    )
    nc.all_engine_barrier()
    assert self.sems is not None
    popped = nc._tile_sem_poison_stack.pop()
    assert popped is self._sem_poison
    nc.clear_and_free_semaphores(list(self.sems.allocated().values()))
    nc.all_engine_barrier()
    _split_waits(nc)


tile.TileContext._drain_and_barrier = _patched_drain_and_barrier

_NC_CACHE = None
RUN_KWARGS: dict = {}
LAST_RES = None


def _build():
    nc = bass.Bass("TRN2", target_bir_lowering=False, debug=False)

    anT_d = nc.dram_tensor("anT", [128, ANT_COLS], BF16, kind="ExternalInput").ap()
    bnT_d = nc.dram_tensor("bnT", [128, N], BF16, kind="ExternalInput").ap()

    acc11_d = nc.dram_tensor("acc11", [128, 8 * TILES], F32, kind="ExternalOutput").ap()
    acc22_d = nc.dram_tensor("acc22", [128, 8 * TILES], F32, kind="ExternalOutput").ap()
    acc12_d = nc.dram_tensor("acc12", [128, 16 * TILES], F32, kind="ExternalOutput").ap()
    band11_d = nc.dram_tensor("band11", [128, R], BF16, kind="ExternalOutput").ap()
    band22_d = nc.dram_tensor("band22", [128, R], BF16, kind="ExternalOutput").ap()
    ca11_d = nc.dram_tensor("ca11", [128, CA_COLS], BF16, kind="ExternalOutput").ap()
    ca22_d = nc.dram_tensor("ca22", [128, CA_COLS], BF16, kind="ExternalOutput").ap()
    ca12_d = nc.dram_tensor("ca12", [128, N], BF16, kind="ExternalOutput").ap()

    with tile.TileContext(nc) as tc:
        with tc.tile_pool(name="pers", bufs=1) as pers:
            anT = pers.tile([128, ANT_COLS], BF16, tag="anT")
            bnT = pers.tile([128, N], BF16, tag="bnT")
            ca12sb = pers.tile([128, (8 - SB12_P0) * PAIR], BF16, tag="ca12sb")
            acc11 = pers.tile([128, 8 * TILES], F32, tag="acc11")
            acc22 = pers.tile([128, 8 * TILES], F32, tag="acc22")
            acc12 = pers.tile([128, 16 * TILES], F32, tag="acc12")

            # input DMAs, chunked so the first window can start early
            for c0, c1 in [(0, 2048), (2048, 4096)] + [
                (x, min(x + 4096, ANT_COLS)) for x in range(4096, ANT_COLS, 4096)
            ]:
                nc.sync.dma_start(anT[:, c0:c1], anT_d[:, c0:c1])
            for c0 in range(0, N, 4096):
                nc.sync.dma_start(bnT[:, c0 : c0 + 4096], bnT_d[:, c0 : c0 + 4096])

            with (
                tc.tile_pool(name="mp", bufs=4, space="PSUM") as mp,
                tc.tile_pool(name="ep", bufs=1) as ep,
                tc.tile_pool(name="scr", bufs=2) as scr,
            ):
                mats = [
                    (anT, anT, acc11, acc11_d, ca11_d, band11_d, True),
                    (bnT, bnT, acc22, acc22_d, ca22_d, band22_d, True),
                    (anT, bnT, acc12, acc12_d, ca12_d, None, False),
                ]
                noff = [0, 0]  # Bresenham state: [sub-chunks seen, offloaded]
                for lhs, rhs, acc, acc_d, ca_d, band_d, sym in mats:
                    nch = 8 if sym else 16
                    for m in range(TILES):
                        lT = lhs[:, m * 128 : (m + 1) * 128]
                        base = m * 128 if sym else 0
                        E4 = None
                        for k in range(nch):
                            c0 = base + k * CHUNK
                            ps = mp.tile([128, CHUNK], F32, tag="mm")
                            for q in range(2):
                                nc.tensor.matmul(
                                    ps[:, q * 512 : (q + 1) * 512],
                                    lT,
                                    rhs[:, c0 + q * 512 : c0 + (q + 1) * 512],
                                )
                            # E tiles pack 8 sub-chunks; colacc reads 2048-wide
                            # pair slices (hw limit for accumulate DMAs)
                            if k % 8 == 0:
                                E4 = ep.tile([128, 8 * CHUNK], BF16, tag="E4", bufs=4)
                            ke = (k % 8) * CHUNK
                            E = E4[:, ke : ke + CHUNK]
                            Ei = E4[:, ke : ke + CHUNK].bitcast(I16)
                            slot = m * nch + k
                            noff[0] += 1
                            off = noff[0] * OFF_NUM // OFF_DEN > noff[1]
                            if off:
                                noff[1] += 1
                                nc.vector.tensor_scalar(
                                    Ei, ps[:], SCHRA_A, SCHRA_B, OP.mult, OP.add
                                )
                                # fast rowsum: identity tensor_scalar keeps DVE 4x
                                # mode (scalar-shaped accum doesn't break it)
                                pscr = scr.tile([128, CHUNK], BF16, tag="pscr")
                                nc.vector.tensor_scalar(
                                    pscr[:],
                                    E,
                                    1.0,
                                    None,
                                    OP.mult,
                                    OP.add,
                                    accum_out=acc[:, slot : slot + 1],
                                )
                            else:
                                nc.scalar.activation(
                                    E,
                                    ps[:],
                                    AF.Exp,
                                    scale=INV_TAU,
                                    accum_out=acc[:, slot : slot + 1],
                                )
                            if k % 2 == 0:
                                continue
                            # pair p = k//2 complete: emit column accumulation
                            p = k // 2
                            pe0 = (p % 4) * PAIR  # pair offset within E4
                            if sym:
                                # region D in [1,63]: cols [m*128+128, m*128+8192)
                                # -> ca idx [m*128, m*128+8064); cols >= prev
                                # tile end (ca idx >= m*128+7936) first-touched
                                pc0 = base + p * PAIR
                                lo = max(pc0, m * 128 + 128)
                                hi = pc0 + PAIR
                                e0 = pe0 + (lo - pc0)
                                a0 = lo - 128
                                a1 = hi - 128
                                new0 = 128 if m == 0 else m * 128 + 8064
                                if lo >= new0:
                                    nc.gpsimd.dma_start(
                                        ca_d[:, a0:a1], E4[:, e0 : pe0 + PAIR]
                                    )
                                elif hi <= new0:
                                    nc.gpsimd.dma_start(
                                        ca_d[:, a0:a1],
                                        E4[:, e0 : pe0 + PAIR],
                                        accum_op=OP.add,
                                    )
                                else:
                                    sp = pe0 + (new0 - pc0)
                                    nc.gpsimd.dma_start(
                                        ca_d[:, a0 : new0 - 128],
                                        E4[:, e0:sp],
                                        accum_op=OP.add,
                                    )
                                    nc.gpsimd.dma_start(
                                        ca_d[:, new0 - 128 : a1],
                                        E4[:, sp : pe0 + PAIR],
                                    )
                            elif p >= SB12_P0:
                                ve = nc.gpsimd if p == SB12_P0 else nc.vector
                                pc0 = (p - SB12_P0) * PAIR
                                if m == 0:
                                    ve.tensor_copy(
                                        ca12sb[:, pc0 : pc0 + PAIR],
                                        E4[:, pe0 : pe0 + PAIR],
                                    )
                                else:
                                    ve.tensor_tensor(
                                        ca12sb[:, pc0 : pc0 + PAIR],
                                        E4[:, pe0 : pe0 + PAIR],
                                        ca12sb[:, pc0 : pc0 + PAIR],
                                        OP.add,
                                    )
                            else:
                                pc0 = p * PAIR
                                if m == 0:
                                    nc.gpsimd.dma_start(
                                        ca_d[:, pc0 : pc0 + PAIR],
                                        E4[:, pe0 : pe0 + PAIR],
                                    )
                                else:
                                    nc.gpsimd.dma_start(
                                        ca_d[:, pc0 : pc0 + PAIR],
                                        E4[:, pe0 : pe0 + PAIR],
                                        accum_op=OP.add,
                                    )
                    if sym:
                        # band pass: distance-64 tiles (m, m+64), rowsum-only,
                        # raw exps exported; host reduces.
                        for h in range(2):
                            ps = mp.tile([128, CHUNK], F32, tag="mm")
                            for j in range(8):
                                m = h * 8 + j
                                nc.tensor.matmul(
                                    ps[:, j * 128 : (j + 1) * 128],
                                    lhs[:, m * 128 : (m + 1) * 128],
                                    rhs[:, WIN + m * 128 : WIN + (m + 1) * 128],
                                )
                            Eb = ep.tile([128, CHUNK], BF16, tag="Eb", bufs=2)
                            nc.scalar.activation(Eb[:], ps[:], AF.Exp, scale=INV_TAU)
                            nc.sync.dma_start(
                                band_d[:, h * CHUNK : (h + 1) * CHUNK], Eb[:]
                            )
                    nc.sync.dma_start(acc_d[:, :], acc[:, :])
                    if not sym:
                        for i in range(8 - SB12_P0):
                            c0 = (SB12_P0 + i) * PAIR
                            nc.sync.dma_start(
                                ca_d[:, c0 : c0 + PAIR],
                                ca12sb[:, i * PAIR : (i + 1) * PAIR],
                            )

    return nc


def _get_nc():
    global _NC_CACHE
    if _NC_CACHE is None:
        _NC_CACHE = _build()
    return _NC_CACHE


def _project(z, W1, b1, W2, b2):
    u = z @ W1 + b1
    h = np.where(u > 0, u, np.expm1(np.minimum(u, 0.0))) @ W2 + b2
    n = np.sqrt(np.sum(h * h, axis=1, keepdims=True))
    return h / np.maximum(n, 1e-12)


def kernel(z1, z2, W1, b1, W2, b2):
    global LAST_RES
    bf = ml_dtypes.bfloat16
    z1 = np.asarray(z1, dtype=np.float32)
    z2 = np.asarray(z2, dtype=np.float32)
    W1 = np.asarray(W1, dtype=np.float32)
    W2 = np.asarray(W2, dtype=np.float32)
    b1 = np.asarray(b1, dtype=np.float32)
    b2 = np.asarray(b2, dtype=np.float32)

    an = _project(z1, W1, b1, W2, b2)
    bn = _project(z2, W1, b1, W2, b2)
    anT_bf = np.ascontiguousarray(an.T).astype(bf)  # [128, N]
    bnT_bf = np.ascontiguousarray(bn.T).astype(bf)

    nc = _get_nc()
    in_maps = []
    for c in range(NCORES):
        a = np.roll(anT_bf, -c * R, axis=1)
        b = np.roll(bnT_bf, -c * R, axis=1)
        in_maps.append(
            {
                "anT": np.ascontiguousarray(a[:, :ANT_COLS]),
                "bnT": np.ascontiguousarray(b),
            }
        )
    res = run_bass_kernel_spmd(nc, in_maps, list(range(NCORES)), **RUN_KWARGS)
    LAST_RES = res

    e2 = np.exp(np.float64(INV_TAU))
    den1 = np.zeros(N, np.float64)
    den2 = np.zeros(N, np.float64)
    idx_ca = None
    for c in range(NCORES):
        r = res.results[c]
        own = slice(c * R, (c + 1) * R)
        # windowed row sums: acc[p, m*nch+k] for row m*128+p
        a11 = r["acc11"].astype(np.float64).reshape(128, TILES, 8)
        a22 = r["acc22"].astype(np.float64).reshape(128, TILES, 8)
        a12 = r["acc12"].astype(np.float64).reshape(128, TILES, 16)
        rs11 = a11.sum(axis=2).T.reshape(R)  # [m,p] -> row m*128+p
        rs22 = a22.sum(axis=2).T.reshape(R)
        rs12 = a12.sum(axis=2).T.reshape(R)
        # band: E[p, m*128+j] = exp tile (m, m+64) -> row m*128+p sums over j
        b11 = r["band11"].astype(np.float64).reshape(128, TILES, 128)
        b22 = r["band22"].astype(np.float64).reshape(128, TILES, 128)
        rs11 += b11.sum(axis=2).T.reshape(R)
        rs22 += b22.sum(axis=2).T.reshape(R)
        den1[own] += rs11 + rs12
        den2[own] += rs22
        # column accumulators: partition-sum then roll to global rows
        cs11 = r["ca11"].astype(np.float64).sum(axis=0)  # local col j+128
        cs22 = r["ca22"].astype(np.float64).sum(axis=0)
        cs12 = r["ca12"].astype(np.float64).sum(axis=0)  # local col j
        if idx_ca is None:
            idx_ca = np.arange(CA_COLS)
        den1[(c * R + 128 + idx_ca) % N] += cs11
        den2[(c * R + 128 + idx_ca) % N] += cs22
        den2[(c * R + np.arange(N)) % N] += cs12
    den1 -= e2
    den2 -= e2

    lognum = 2.0 * np.sum(an.astype(np.float64) * bn.astype(np.float64), axis=1)
    loss = np.mean(0.5 * (np.log(den1) + np.log(den2)) - lognum)
    return np.array(loss, dtype=np.float32)


# revision 31
# speedup vs baseline: 1.1599x; 1.0135x over previous
"""Trainium2 Bass kernel for the HGCA contrastive loss (nn_HGCA_10857677324785).

loss = mean_i 0.5*(l1_i + l2_i),
  l1_i = log(den1_i) - log(num_i), den1_i = sum_j e^{2 an_i.an_j} + sum_j e^{2 an_i.bn_j} - e^2
  l2_i = log(den2_i) - log(num_i), den2_i = sum_j e^{2 bn_i.bn_j} + sum_j e^{2 bn_i.an_j} - e^2
where an/bn are the L2-normalized projections elu-MLP(z1/z2).

Distribution: the projections (O(N D^2), 0.5% of the FLOPs) are computed on
the host in f32 — the sharding hint's "all-gathered normalized projections"
— and handed to every core pre-transposed ([d, i] layout, bf16) and
row-rolled so each core's 2048 rows sit at local columns [0, 2048).

Each core computes its row-block of the three N x N similarity exps.  The
symmetric matrices S11 = an@an.T and S22 = bn@bn.T are only half-computed:
each 128-row tile m processes a diagonal-anchored window of 64 column tiles
(local cols [m*128, m*128+8192)) plus the distance-64 tile as a separate
"band" pass.  Row sums come from the ACT engine's fused accumulator; column
sums of the D in [1,63] part are accumulated in bf16 on the DVE and exported
raw — by symmetry they are exactly the row-sum contributions of the
uncomputed distance >= 65 tiles.  S12 is not symmetric: full rows with both
row sums (ACT accum) and bf16 column accumulation (DVE).  The host sums the
raw column accumulators over partitions, rolls them into global row space,
and assembles the scalar loss in f64 (log num_i = 2 an_i.bn_i directly).
"""

import ml_dtypes
import numpy as np

import concourse.bass as bass
import concourse.tile as tile
from concourse import mybir
from concourse.bass_utils import run_bass_kernel_spmd

N = 16384
D = 128
NCORES = 8
R = N // NCORES  # 2048 rows per core
TILES = R // 128  # 16 row tiles per core
WIN = 8192  # window: distance tiles 0..63
CHUNK = 1024  # psum/exp sub-chunk width (2-bank PSUM tiles, 4-deep rotation)
PAIR = 2048  # column-accumulate granularity (hw limit for accum DMAs)
CA_COLS = 15 * 128 + WIN - 128  # 9984: colacc for D in [1,63]
ANT_COLS = 15 * 128 + WIN + 2048  # 10240: rightmost anT column ever read
INV_TAU = 2.0  # 1/0.5
F32 = mybir.dt.float32
BF16 = mybir.dt.bfloat16
I16 = mybir.dt.int16
AF = mybir.ActivationFunctionType
OP = mybir.AluOpType

# Schraudolph fast-exp on the DVE: I = int16(A*s + B); bf16-bits(I) ~ exp(2s).
# A folds in 1/tau; B calibrated for zero mean multiplicative bias over the
# (near-uniform) mantissa phase.  Offloads ACT-engine exp work per chunk.
SCHRA_A = 2.0 * 128.0 / float(np.log(2.0))
SCHRA_B = 16250.0
OFF_NUM, OFF_DEN = 223, 512  # fraction of sub-chunks exp'd on DVE (Bresenham)
SB12_P0 = 6  # S12 pairs p >= this accumulate in SBUF (gpsimd p=6, DVE p=7);
# pairs p < 6 go per-pair SWDGE-accumulate to DRAM.  SBUF pairs sit last so
# the kernel tail is a short HWDGE export, not a chained SWDGE accumulate.

# This walrus build supports at most 2 sync waits per instruction; Tile's sem
# assignment freely emits 3-11. Post-pass: hoist excess waits onto injected
# same-engine EventSemaphore fillers (engine queues are FIFO, so waits on an
# earlier filler happen-before the original instruction executes).

_MAX_WAITS = 1


def _split_waits(nc):
    for fn in nc.m.functions:
        for bb in fn.blocks:
            insts = list(bb.instructions)
            out = []
            changed = False
            for inst in insts:
                si = inst.sync_info
                w = list(si.on_wait) if si and si.on_wait else []
                if len(w) > _MAX_WAITS:
                    changed = True
                    extra, keep = w[:-_MAX_WAITS], w[-_MAX_WAITS:]
                    for i in range(0, len(extra), _MAX_WAITS):
                        f = mybir.InstEventSemaphore(
                            name=f"{inst.name}_wsplit{i}",
                            engine=inst.engine,
                            ins=[],
                            outs=[],
                            sync_info=mybir.SyncInfo(
                                on_wait=extra[i : i + _MAX_WAITS], on_update=[]
                            ),
                        )
                        out.append(f)
                    inst.sync_info = mybir.SyncInfo(
                        on_wait=keep,
                        on_update=list(si.on_update) if si.on_update else [],
                    )
                out.append(inst)
            if changed:
                bb.instructions = out


def _patched_drain_and_barrier(self, tick_clock, wait_clock):
    from concourse.vector_clock import ScopedClock

    nc = self.nc
    drain_inst = nc.sync.drain()
    wait_clock.add_sem_waits(
        drain_inst.ins, ScopedClock({None: tick_clock.global_clock})
    )
    nc.all_engine_barrier()
    assert self.sems is not None
    popped = nc._tile_sem_poison_stack.pop()
    assert popped is self._sem_poison
    nc.clear_and_free_semaphores(list(self.sems.allocated().values()))
    nc.all_engine_barrier()
    _split_waits(nc)


tile.TileContext._drain_and_barrier = _patched_drain_and_barrier

_NC_CACHE = None
RUN_KWARGS: dict = {}
LAST_RES = None


def _build():
    nc = bass.Bass("TRN2", target_bir_lowering=False, debug=False)

    anT_d = nc.dram_tensor("anT", [128, ANT_COLS], BF16, kind="ExternalInput").ap()
    bnT_d = nc.dram_tensor("bnT", [128, N], BF16, kind="ExternalInput").ap()

    acc11_d = nc.dram_tensor("acc11", [128, 8 * TILES], F32, kind="ExternalOutput").ap()
    acc22_d = nc.dram_tensor("acc22", [128, 8 * TILES], F32, kind="ExternalOutput").ap()
    acc12_d = nc.dram_tensor("acc12", [128, 16 * TILES], F32, kind="ExternalOutput").ap()
    band11_d = nc.dram_tensor("band11", [128, R], BF16, kind="ExternalOutput").ap()
    band22_d = nc.dram_tensor("band22", [128, R], BF16, kind="ExternalOutput").ap()
    ca11_d = nc.dram_tensor("ca11", [128, CA_COLS], BF16, kind="ExternalOutput").ap()
    ca22_d = nc.dram_tensor("ca22", [128, CA_COLS], BF16, kind="ExternalOutput").ap()
    ca12_d = nc.dram_tensor("ca12", [128, N], BF16, kind="ExternalOutput").ap()

    with tile.TileContext(nc) as tc:
        with tc.tile_pool(name="pers", bufs=1) as pers:
            anT = pers.tile([128, ANT_COLS], BF16, tag="anT")
            bnT = pers.tile([128, N], BF16, tag="bnT")
            ca12sb = pers.tile([128, (8 - SB12_P0) * PAIR], BF16, tag="ca12sb")
            acc11 = pers.tile([128, 8 * TILES], F32, tag="acc11")
            acc22 = pers.tile([128, 8 * TILES], F32, tag="acc22")
            acc12 = pers.tile([128, 16 * TILES], F32, tag="acc12")

            # input DMAs, chunked so the first window can start early
            for c0, c1 in [(0, 2048), (2048, 4096)] + [
                (x, min(x + 4096, ANT_COLS)) for x in range(4096, ANT_COLS, 4096)
            ]:
                nc.sync.dma_start(anT[:, c0:c1], anT_d[:, c0:c1])
            for c0 in range(0, N, 4096):
                nc.sync.dma_start(bnT[:, c0 : c0 + 4096], bnT_d[:, c0 : c0 + 4096])

            with (
                tc.tile_pool(name="mp", bufs=4, space="PSUM") as mp,
                tc.tile_pool(name="ep", bufs=1) as ep,
                tc.tile_pool(name="scr", bufs=2) as scr,
            ):
                mats = [
                    (anT, anT, acc11, acc11_d, ca11_d, band11_d, True),
                    (bnT, bnT, acc22, acc22_d, ca22_d, band22_d, True),
                    (anT, bnT, acc12, acc12_d, ca12_d, None, False),
                ]
                noff = [0, 0]  # Bresenham state: [sub-chunks seen, offloaded]
                for lhs, rhs, acc, acc_d, ca_d, band_d, sym in mats:
                    nch = 8 if sym else 16
                    for m in range(TILES):
                        lT = lhs[:, m * 128 : (m + 1) * 128]
                        base = m * 128 if sym else 0
                        E4 = None
                        for k in range(nch):
                            c0 = base + k * CHUNK
                            ps = mp.tile([128, CHUNK], F32, tag="mm")
                            for q in range(2):
                                nc.tensor.matmul(
                                    ps[:, q * 512 : (q + 1) * 512],
                                    lT,
                                    rhs[:, c0 + q * 512 : c0 + (q + 1) * 512],
                                )
                            # E tiles pack 8 sub-chunks; colacc reads 2048-wide
                            # pair slices (hw limit for accumulate DMAs)
                            if k % 8 == 0:
                                E4 = ep.tile([128, 8 * CHUNK], BF16, tag="E4", bufs=5)
                            ke = (k % 8) * CHUNK
                            E = E4[:, ke : ke + CHUNK]
                            Ei = E4[:, ke : ke + CHUNK].bitcast(I16)
                            slot = m * nch + k
                            noff[0] += 1
                            off = noff[0] * OFF_NUM // OFF_DEN > noff[1]
                            if off:
                                noff[1] += 1
                                nc.vector.tensor_scalar(
                                    Ei, ps[:], SCHRA_A, SCHRA_B, OP.mult, OP.add
                                )
                                # fast rowsum: identity tensor_scalar keeps DVE 4x
                                # mode (scalar-shaped accum doesn't break it)
                                pscr = scr.tile([128, CHUNK], BF16, tag="pscr")
                                nc.vector.tensor_scalar(
                                    pscr[:],
                                    E,
                                    1.0,
                                    None,
                                    OP.mult,
                                    OP.add,
                                    accum_out=acc[:, slot : slot + 1],
                                )
                            else:
                                nc.scalar.activation(
                                    E,
                                    ps[:],
                                    AF.Exp,
                                    scale=INV_TAU,
                                    accum_out=acc[:, slot : slot + 1],
                                )
                            if k % 2 == 0:
                                continue
                            # pair p = k//2 complete: emit column accumulation
                            p = k // 2
                            pe0 = (p % 4) * PAIR  # pair offset within E4
                            if sym:
                                # region D in [1,63]: cols [m*128+128, m*128+8192)
                                # -> ca idx [m*128, m*128+8064); cols >= prev
                                # tile end (ca idx >= m*128+7936) first-touched
                                pc0 = base + p * PAIR
                                lo = max(pc0, m * 128 + 128)
                                hi = pc0 + PAIR
                                e0 = pe0 + (lo - pc0)
                                a0 = lo - 128
                                a1 = hi - 128
                                new0 = 128 if m == 0 else m * 128 + 8064
                                if lo >= new0:
                                    nc.gpsimd.dma_start(
                                        ca_d[:, a0:a1], E4[:, e0 : pe0 + PAIR]
                                    )
                                elif hi <= new0:
                                    nc.gpsimd.dma_start(
                                        ca_d[:, a0:a1],
                                        E4[:, e0 : pe0 + PAIR],
                                        accum_op=OP.add,
                                    )
                                else:
                                    sp = pe0 + (new0 - pc0)
                                    nc.gpsimd.dma_start(
                                        ca_d[:, a0 : new0 - 128],
                                        E4[:, e0:sp],
                                        accum_op=OP.add,
                                    )
                                    nc.gpsimd.dma_start(
                                        ca_d[:, new0 - 128 : a1],
                                        E4[:, sp : pe0 + PAIR],
                                    )
                            elif p >= SB12_P0:
                                ve = nc.gpsimd if p == SB12_P0 else nc.vector
                                pc0 = (p - SB12_P0) * PAIR
                                if m == 0:
                                    ve.tensor_copy(
                                        ca12sb[:, pc0 : pc0 + PAIR],
                                        E4[:, pe0 : pe0 + PAIR],
                                    )
                                else:
                                    ve.tensor_tensor(
                                        ca12sb[:, pc0 : pc0 + PAIR],
                                        E4[:, pe0 : pe0 + PAIR],
                                        ca12sb[:, pc0 : pc0 + PAIR],
                                        OP.add,
                                    )
                            else:
                                pc0 = p * PAIR
                                if m == 0:
                                    nc.gpsimd.dma_start(
                                        ca_d[:, pc0 : pc0 + PAIR],
                                        E4[:, pe0 : pe0 + PAIR],
                                    )
                                else:
                                    nc.gpsimd.dma_start(
                                        ca_d[:, pc0 : pc0 + PAIR],
                                        E4[:, pe0 : pe0 + PAIR],
                                        accum_op=OP.add,
                                    )
                    if sym:
                        # band pass: distance-64 tiles (m, m+64), rowsum-only,
                        # raw exps exported; host reduces.
                        for h in range(2):
                            ps = mp.tile([128, CHUNK], F32, tag="mm")
                            for j in range(8):
                                m = h * 8 + j
                                nc.tensor.matmul(
                                    ps[:, j * 128 : (j + 1) * 128],
                                    lhs[:, m * 128 : (m + 1) * 128],
                                    rhs[:, WIN + m * 128 : WIN + (m + 1) * 128],
                                )
                            Eb = ep.tile([128, CHUNK], BF16, tag="Eb", bufs=2)
                            nc.scalar.activation(Eb[:], ps[:], AF.Exp, scale=INV_TAU)
                            nc.sync.dma_start(
                                band_d[:, h * CHUNK : (h + 1) * CHUNK], Eb[:]
                            )
                    nc.sync.dma_start(acc_d[:, :], acc[:, :])
                    if not sym:
                        for i in range(8 - SB12_P0):
                            c0 = (SB12_P0 + i) * PAIR
                            nc.sync.dma_start(
                                ca_d[:, c0 : c0 + PAIR],
                                ca12sb[:, i * PAIR : (i + 1) * PAIR],
                            )

    return nc


def _get_nc():
    global _NC_CACHE
    if _NC_CACHE is None:
        _NC_CACHE = _build()
    return _NC_CACHE


def _project(z, W1, b1, W2, b2):
    u = z @ W1 + b1
    h = np.where(u > 0, u, np.expm1(np.minimum(u, 0.0))) @ W2 + b2
    n = np.sqrt(np.sum(h * h, axis=1, keepdims=True))
    return h / np.maximum(n, 1e-12)


def kernel(z1, z2, W1, b1, W2, b2):
    global LAST_RES
    bf = ml_dtypes.bfloat16
    z1 = np.asarray(z1, dtype=np.float32)
    z2 = np.asarray(z2, dtype=np.float32)
    W1 = np.asarray(W1, dtype=np.float32)
    W2 = np.asarray(W2, dtype=np.float32)
    b1 = np.asarray(b1, dtype=np.float32)
    b2 = np.asarray(b2, dtype=np.float32)

    an = _project(z1, W1, b1, W2, b2)
    bn = _project(z2, W1, b1, W2, b2)
    anT_bf = np.ascontiguousarray(an.T).astype(bf)  # [128, N]
    bnT_bf = np.ascontiguousarray(bn.T).astype(bf)

    nc = _get_nc()
    in_maps = []
    for c in range(NCORES):
        a = np.roll(anT_bf, -c * R, axis=1)
        b = np.roll(bnT_bf, -c * R, axis=1)
        in_maps.append(
            {
                "anT": np.ascontiguousarray(a[:, :ANT_COLS]),
                "bnT": np.ascontiguousarray(b),
            }
        )
    res = run_bass_kernel_spmd(nc, in_maps, list(range(NCORES)), **RUN_KWARGS)
    LAST_RES = res

    e2 = np.exp(np.float64(INV_TAU))
    den1 = np.zeros(N, np.float64)
    den2 = np.zeros(N, np.float64)
    idx_ca = None
    for c in range(NCORES):
        r = res.results[c]
        own = slice(c * R, (c + 1) * R)
        # windowed row sums: acc[p, m*nch+k] for row m*128+p
        a11 = r["acc11"].astype(np.float64).reshape(128, TILES, 8)
        a22 = r["acc22"].astype(np.float64).reshape(128, TILES, 8)
        a12 = r["acc12"].astype(np.float64).reshape(128, TILES, 16)
        rs11 = a11.sum(axis=2).T.reshape(R)  # [m,p] -> row m*128+p
        rs22 = a22.sum(axis=2).T.reshape(R)
        rs12 = a12.sum(axis=2).T.reshape(R)
        # band: E[p, m*128+j] = exp tile (m, m+64) -> row m*128+p sums over j
        b11 = r["band11"].astype(np.float64).reshape(128, TILES, 128)
        b22 = r["band22"].astype(np.float64).reshape(128, TILES, 128)
        rs11 += b11.sum(axis=2).T.reshape(R)
        rs22 += b22.sum(axis=2).T.reshape(R)
        den1[own] += rs11 + rs12
        den2[own] += rs22
        # column accumulators: partition-sum then roll to global rows
        cs11 = r["ca11"].astype(np.float64).sum(axis=0)  # local col j+128
        cs22 = r["ca22"].astype(np.float64).sum(axis=0)
        cs12 = r["ca12"].astype(np.float64).sum(axis=0)  # local col j
        if idx_ca is None:
            idx_ca = np.arange(CA_COLS)
        den1[(c * R + 128 + idx_ca) % N] += cs11
        den2[(c * R + 128 + idx_ca) % N] += cs22
        den2[(c * R + np.arange(N)) % N] += cs12
    den1 -= e2
    den2 -= e2

    lognum = 2.0 * np.sum(an.astype(np.float64) * bn.astype(np.float64), axis=1)
    loss = np.mean(0.5 * (np.log(den1) + np.log(den2)) - lognum)
    return np.array(loss, dtype=np.float32)
